# revision 1
# baseline (speedup 1.0000x reference)
"""Causal self-attention (B=4, T=2048, C=1024, H=16) on 8 trn2 NeuronCores.

Sharding: core c -> (batch b = c//2, head-group g = c%2 of 8 heads).
Each core computes qkv projection, causal attention and the proj partial-sum
for its 8 heads on its batch; the host sums the two head-group partials per
batch (row-parallel linear unshard).

Per-core kernel layout (all on-device matmuls bf16, f32 accumulation):
  xT [C, T] (pre-transposed on host) so QKV contraction runs with c on the
  partition axis with zero on-device transposes.
  QT/KT [2*64, T] per head pair -> scores S_T[t_k, t_q] via two k=64 matmuls
  packed into PE row-groups 0-63/64-127 (tile_position auto-derivation).
  Softmax without max-subtraction (logits ~ N(0,1), fp32-safe); denominator
  via an appended ones-column in the AV lhsT (m=65, row 64 = sum of exp).
  exp on ScalarE in [128, 2048] mega-tiles (bf16 PSUM) to amortize overhead.
  Normalization: reciprocal_approx_fast of denoms + GpSimd partition
  broadcast + one in-place multiply per pair; proj with k=128 chunks.
"""

from contextlib import ExitStack

import ml_dtypes
import numpy as np
import orjson

import concourse.bass as bass
import concourse.mybir as mybir
import concourse.tile as tile
from concourse.bass_utils import run_bass_kernel_spmd

BF16 = mybir.dt.bfloat16
F32 = mybir.dt.float32
F32R = mybir.dt.float32r
AF = mybir.ActivationFunctionType

T, C, H, DH = 2048, 1024, 16, 64
NCORES = 8
NPAIR = 4            # head pairs per core (8 heads)
CCH = C // 128       # contraction chunks for qkv
TQ = T // 512        # query chunks
NT = T // 128        # token tiles
VROW = 65            # 64 v-cols + ones column

# --- walrus in this env accepts only ONE sync-wait per instruction: split
# extras onto preceding same-engine NoOps at the BIR-JSON level.
if not getattr(bass.Bass, "_ant_wait_split", False):
    _orig_to_json_bytes = bass.Bass.to_json_bytes

    def _to_json_split_waits(self):
        m = orjson.loads(_orig_to_json_bytes(self))
        for f in m.get("functions", []):
            for bb in f.get("blocks") or []:
                insts = bb.get("instructions") or []
                out, changed = [], False
                for inst in insts:
                    si = inst.get("sync_info")
                    waits = (si or {}).get("on_wait") or []
                    if len(waits) > 1:
                        for j, w in enumerate(waits[:-1]):
                            out.append({
                                "debug": inst.get("debug", 0),
                                "engine": inst["engine"],
                                "ins": [], "outs": [],
                                "name": f"{inst['name']}-sw{j}",
                                "opcode": "NoOp",
                                "sync_info": {"on_wait": [w], "on_update": []},
                            })
                        si["on_wait"] = waits[-1:]
                        changed = True
                    out.append(inst)
                if changed:
                    bb["instructions"] = out
        return orjson.dumps(m)

    bass.Bass.to_json_bytes = _to_json_split_waits
    bass.Bass._ant_wait_split = True


def build_program() -> bass.Bass:
    nc = bass.Bass()
    xT = nc.dram_tensor("xT", [C, T], BF16, kind="ExternalInput")
    wqkvT = nc.dram_tensor("wqkvT", [C, 1536], BF16, kind="ExternalInput")
    wpT = nc.dram_tensor("wpT", [512, C], BF16, kind="ExternalInput")
    dmask = nc.dram_tensor("dmask", [128, 2048], BF16, kind="ExternalInput")
    seld = nc.dram_tensor("sel", [2, 128], F32R, kind="ExternalInput")
    out = nc.dram_tensor("out", [T, C], F32, kind="ExternalOutput")

    with ExitStack() as ctx:
        tc = ctx.enter_context(tile.TileContext(nc))
        const = ctx.enter_context(tc.tile_pool(name="const", bufs=1))
        pss = ctx.enter_context(tc.tile_pool(name="pss", bufs=2, space="PSUM"))
        psy = ctx.enter_context(tc.tile_pool(name="psy", bufs=2, space="PSUM"))
        ppool = ctx.enter_context(tc.tile_pool(name="ppool", bufs=3))
        spool = ctx.enter_context(tc.tile_pool(name="spool", bufs=2))
        rbpool = ctx.enter_context(tc.tile_pool(name="rbpool", bufs=2))
        opool = ctx.enter_context(tc.tile_pool(name="opool", bufs=2))
        dram = ctx.enter_context(tc.tile_pool(name="dram", bufs=1, space="DRAM"))
        dstage = dram.tile([16, 1024], F32, tag="dstage")
        rstage = dram.tile([16, 1024], F32, tag="rstage")

        xT_sb = const.tile([128, CCH, T], BF16, tag="xT")
        wq_sb = const.tile([128, CCH, 1536], BF16, tag="wq")
        wp_sb = const.tile([128, 4, C], BF16, tag="wp")
        dm_sb = const.tile([128, 2048], BF16, tag="dm")
        QT_sb = const.tile([128, NPAIR, T], BF16, tag="QT")
        KT_sb = const.tile([128, NPAIR, T], BF16, tag="KT")
        V_sb = const.tile([128, NT, 8 * VROW], BF16, tag="V")
        Yu_sb = const.tile([128, NPAIR, T], BF16, tag="Yu")

        for c in range(CCH):
            # split halves across DMA queues for a faster input ramp
            nc.sync.dma_start(xT_sb[:, c, 0:1024], xT[c * 128:(c + 1) * 128, 0:1024])
            nc.sync.dma_start(xT_sb[:, c, 1024:2048], xT[c * 128:(c + 1) * 128, 1024:2048])
            nc.sync.dma_start(wq_sb[:, c, 0:768], wqkvT[c * 128:(c + 1) * 128, 0:768])
            nc.sync.dma_start(wq_sb[:, c, 768:1536], wqkvT[c * 128:(c + 1) * 128, 768:1536])
        for c in range(4):
            nc.sync.dma_start(wp_sb[:, c, :], wpT[c * 128:(c + 1) * 128, :])
        nc.sync.dma_start(dm_sb[:], dmask[:])

        vr = V_sb[:].rearrange("p n (h e) -> p n h e", e=VROW)
        nc.gpsimd.memset(vr[:, :, :, 64:65], 1.0)
        # selector for the k=2 reciprocal-broadcast matmul:
        # out[m,:] = sel[0,m]*rt[0,:] + sel[1,m]*rt[1,:] -> A rows 0-63, B rows 64-127
        sel_sb = const.tile([128, 128], F32R, tag="sel")
        nc.sync.dma_start(sel_sb[0:2, :], seld[:])

        # ---------------- QKV projection ----------------
        for pair in range(NPAIR):
            for q in range(TQ):
                for colbase, dst in ((0, QT_sb), (512, KT_sb)):
                    ps = pss.tile([128, 512], F32, tag="ss")
                    for c in range(CCH):
                        nc.tensor.matmul(
                            ps[:],
                            wq_sb[:, c, colbase + pair * 128: colbase + (pair + 1) * 128],
                            xT_sb[:, c, q * 512:(q + 1) * 512],
                            start=(c == 0), stop=(c == CCH - 1),
                        )
                    nc.scalar.copy(dst[:, pair, q * 512:(q + 1) * 512], ps[:])
        for tt in range(NT):
            ps = pss.tile([128, 512], F32, tag="ss")
            for c in range(CCH):
                nc.tensor.matmul(
                    ps[:],
                    xT_sb[:, c, tt * 128:(tt + 1) * 128],
                    wq_sb[:, c, 1024:1536],
                    start=(c == 0), stop=(c == CCH - 1),
                )
            nc.scalar.copy(
                vr[:, tt, :, 0:64],
                ps[:].rearrange("p (h d) -> p h d", d=64),
            )

        # ---------------- attention ----------------
        for pair in range(NPAIR):
            hA, hB = 2 * pair, 2 * pair + 1
            for q in range(TQ):
                ya = psy.tile([VROW, 512], F32, tag="yA")
                yb = psy.tile([VROW, 512], F32, tag="yB")
                ntk = 4 * (q + 1)
                for tk in range(ntk):
                    ssm = pss.tile([128, 1024], F32, tag="ss")
                    nc.tensor.matmul(
                        ssm[:, 0:512],
                        KT_sb[0:64, pair, tk * 128:(tk + 1) * 128],
                        QT_sb[0:64, pair, q * 512:(q + 1) * 512],
                        start=True, stop=True,
                    )
                    nc.tensor.matmul(
                        ssm[:, 512:1024],
                        KT_sb[64:128, pair, tk * 128:(tk + 1) * 128],
                        QT_sb[64:128, pair, q * 512:(q + 1) * 512],
                        start=True, stop=True,
                    )
                    pm = ppool.tile([128, 1024], BF16, tag="P")
                    nc.scalar.activation(pm[:], ssm[:], AF.Exp)
                    if tk >= 4 * q:
                        off = (tk - 4 * q) * 512
                        nc.vector.tensor_mul(
                            pm[:, 0:512], pm[:, 0:512], dm_sb[:, off:off + 512])
                        nc.vector.tensor_mul(
                            pm[:, 512:1024], pm[:, 512:1024], dm_sb[:, off:off + 512])
                    first = (tk == 0)
                    last = (tk == ntk - 1)
                    nc.tensor.matmul(
                        ya[:], V_sb[:, tk, hA * VROW:(hA + 1) * VROW],
                        pm[:, 0:512],
                        start=first, stop=last,
                    )
                    nc.tensor.matmul(
                        yb[:], V_sb[:, tk, hB * VROW:(hB + 1) * VROW],
                        pm[:, 512:1024],
                        start=first, stop=last,
                    )
                # epilogue: strip denominators to DRAM staging, evac y
                dt = spool.tile([128, 1024], F32, tag="Dt")
                nc.vector.tensor_copy(dt[64:65, 0:512], ya[64:65, :])
                nc.vector.tensor_copy(dt[64:65, 512:1024], yb[64:65, :])
                nc.sync.dma_start(dstage[4 * pair + q, :], dt[64:65, :])
                nc.vector.tensor_copy(
                    Yu_sb[0:64, pair, q * 512:(q + 1) * 512], ya[0:64, :])
                bs = spool.tile([64, 512], BF16, tag="Bs")
                nc.vector.tensor_copy(bs[:], yb[0:64, :])
                nc.sync.dma_start(
                    Yu_sb[64:128, pair, q * 512:(q + 1) * 512], bs[:])
            # batched reciprocal of this pair's 4096 denominators ([128, 32])
            dp = spool.tile([128, 32], F32, tag="Dp")
            nc.sync.dma_start(
                dp[:],
                dstage[4 * pair:4 * pair + 4, :]
                .rearrange("q v -> (q v)").rearrange("(r c) -> r c", c=32))
            rp = spool.tile([128, 32], F32, tag="Rp")
            nc.vector.reciprocal(rp[:], dp[:])
            nc.sync.dma_start(
                rstage[4 * pair:4 * pair + 4, :]
                .rearrange("q v -> (q v)").rearrange("(r c) -> r c", c=32),
                rp[:])
        # normalize: k=2 PE broadcast of reciprocals + one in-place mul per pair
        # (kept out of the attention loop so the DMA/recip chain never blocks
        # the in-order PE stream between pairs)
        for pair in range(NPAIR):
            RB = rbpool.tile([128, T], F32, tag="RB")
            for q in range(TQ):
                rt = spool.tile([128, 512], F32R, tag="Rt")
                nc.sync.dma_start(
                    rt[0:2, :],
                    rstage[4 * pair + q, :]
                    .rearrange("(a c) -> a c", c=512).bitcast(F32R))
                bc = pss.tile([128, 512], F32, tag="ss")
                nc.tensor.matmul(
                    bc[:], sel_sb[0:2, :], rt[0:2, :], start=True, stop=True)
                nc.vector.tensor_copy(RB[:, q * 512:(q + 1) * 512], bc[:])
            nc.vector.tensor_mul(Yu_sb[:, pair, :], Yu_sb[:, pair, :], RB[:])

        # ---------------- output projection (partial over this core's heads) --
        for tt in range(NT):
            ot = opool.tile([128, C], F32, tag="Ot")
            for oc in range(2):
                po = pss.tile([128, 512], F32, tag="ss")
                for pair in range(NPAIR):
                    nc.tensor.matmul(
                        po[:],
                        Yu_sb[:, pair, tt * 128:(tt + 1) * 128],
                        wp_sb[:, pair, oc * 512:(oc + 1) * 512],
                        start=(pair == 0), stop=(pair == NPAIR - 1),
                    )
                nc.vector.tensor_copy(ot[:, oc * 512:(oc + 1) * 512], po[:])
            nc.sync.dma_start(out[tt * 128:(tt + 1) * 128, :], ot[:])

    return nc


def make_in_maps(x: np.ndarray, w_qkv: np.ndarray, w_proj: np.ndarray):
    bf = ml_dtypes.bfloat16
    scale = np.float32(DH ** -0.5)

    iq = np.arange(512)[None, :]
    ik = np.arange(128)[:, None]
    dmask = np.concatenate(
        [(iq >= j * 128 + ik) for j in range(4)], axis=1).astype(bf)

    in_maps = []
    for core in range(NCORES):
        b, g = core // 2, core % 2
        xTb = np.ascontiguousarray(x[b].T).astype(bf)           # [C, T]
        wq = (w_qkv[512 * g: 512 * g + 512] * scale).astype(np.float32)
        wk = w_qkv[1024 + 512 * g: 1024 + 512 * g + 512]
        wv = w_qkv[2048 + 512 * g: 2048 + 512 * g + 512]
        wqkvT = np.ascontiguousarray(
            np.concatenate([wq, wk, wv], axis=0).T).astype(bf)  # [C, 1536]
        wpT = np.ascontiguousarray(
            w_proj[:, 512 * g: 512 * g + 512].T).astype(bf)     # [512, C]
        sel = np.zeros((2, 128), dtype=np.float32)
        sel[0, 0:64] = 1.0
        sel[1, 64:128] = 1.0
        in_maps.append({"xT": xTb, "wqkvT": wqkvT, "wpT": wpT, "dmask": dmask,
                        "sel": sel})
    return in_maps


_NC = None


def kernel(x: np.ndarray, w_qkv: np.ndarray, w_proj: np.ndarray,
           _trace: bool = False, _return_raw: bool = False) -> np.ndarray:
    global _NC
    x = np.asarray(x, dtype=np.float32)
    w_qkv = np.asarray(w_qkv, dtype=np.float32)
    w_proj = np.asarray(w_proj, dtype=np.float32)
    if _NC is None:
        _NC = build_program()
    in_maps = make_in_maps(x, w_qkv, w_proj)
    res = run_bass_kernel_spmd(_NC, in_maps, list(range(NCORES)), trace=_trace)
    B = x.shape[0]
    outp = np.empty((B, T, C), dtype=np.float32)
    for b in range(B):
        outp[b] = res.results[2 * b]["out"] + res.results[2 * b + 1]["out"]
    if _return_raw:
        return outp, res
    return outp



# revision 44
# speedup vs baseline: 1.3119x; 1.3119x over previous
"""Causal self-attention (B=4, T=2048, C=1024, H=16) on 8 trn2 NeuronCores.

Sharding: core c -> (batch b = c//2, head-group g = c%2 of 8 heads).
Each core computes qkv projection, causal attention and the proj partial-sum
for its 8 heads on its batch; the host sums the two head-group partials per
batch (row-parallel linear unshard).

Per-core kernel (all PE matmuls bf16, f32 accumulation):
  Scores S^T[k,q] at exact causal q128 granularity, two heads of a pair
  row-group-packed (partitions 0-63 / 64-127).
  exp on ScalarE over pair-packed [128, 2w] PSUM spans.
  AV transposed: y[q,d] = sum_k P^T[k,q].T V[k,d] streams V (n=65 incl. a
  ones column whose output column is the softmax denominator, landing on
  the q partition axis).
  Normalization: DVE reciprocal of the PSUM denominator column + one
  per-partition-scalar multiply fused into the y eviction.
  y pairs transposed back with one PE-transpose per (pair, qtile) into
  Y^T[c, t] for the output projection (k=128 chunks over the 4 pairs).
  DMA order + chunk-major "quad" QKV accumulation pace the input ramp so PE
  starts ~2us in; attention for pairs 0/1 starts once their Q/K q-halves
  are done, with remaining QKV jobs as PE fillers between attention steps.

wqkvT host column layout: [Qp0|Kp0|Qp1|Kp1|Qp2|Kp2|Qp3|Kp3|V] (128 each,
V = 512) so each DMA piece feeds a contiguous phase.
"""

from contextlib import ExitStack

import ml_dtypes
import numpy as np
import orjson

import concourse.bass as bass
import concourse.mybir as mybir
import concourse.tile as tile
from concourse.bass_utils import run_bass_kernel_spmd

BF16 = mybir.dt.bfloat16
F32 = mybir.dt.float32
AF = mybir.ActivationFunctionType

T, C, H, DH = 2048, 1024, 16, 64
NCORES = 8
NPAIR = 4            # head pairs per core (8 heads)
CCH = C // 128       # contraction chunks for qkv
NT = T // 128        # 128-token tiles (k-tiles, q-tiles, t-tiles)
VW = 65              # 64 v-cols + ones column

# --- walrus in this env accepts only ONE sync-wait per instruction: split
# extras onto preceding same-engine NoOps at the BIR-JSON level.
if not getattr(bass.Bass, "_ant_wait_split", False):
    _orig_to_json_bytes = bass.Bass.to_json_bytes

    def _to_json_split_waits(self):
        m = orjson.loads(_orig_to_json_bytes(self))
        for f in m.get("functions", []):
            for bb in f.get("blocks") or []:
                insts = bb.get("instructions") or []
                out, changed = [], False
                for inst in insts:
                    si = inst.get("sync_info")
                    waits = (si or {}).get("on_wait") or []
                    if len(waits) > 1:
                        for j, w in enumerate(waits[:-1]):
                            out.append({
                                "debug": inst.get("debug", 0),
                                "engine": inst["engine"],
                                "ins": [], "outs": [],
                                "name": f"{inst['name']}-sw{j}",
                                "opcode": "NoOp",
                                "sync_info": {"on_wait": [w], "on_update": []},
                            })
                        si["on_wait"] = waits[-1:]
                        changed = True
                    out.append(inst)
                if changed:
                    bb["instructions"] = out
        return orjson.dumps(m)

    bass.Bass.to_json_bytes = _to_json_split_waits
    bass.Bass._ant_wait_split = True


def qcol(p):
    return 256 * p


def kcol(p):
    return 256 * p + 128


JOBLOG = []
BISECT = set()


def build_program() -> bass.Bass:
    JOBLOG.clear()
    nc = bass.Bass()
    xT = nc.dram_tensor("xT", [C, T], BF16, kind="ExternalInput")
    wqkvT = nc.dram_tensor("wqkvT", [C, 1536], BF16, kind="ExternalInput")
    wpT = nc.dram_tensor("wpT", [512, C], BF16, kind="ExternalInput")
    dmask = nc.dram_tensor("dmask", [128, 128], BF16, kind="ExternalInput")
    ident = nc.dram_tensor("ident", [128, 128], BF16, kind="ExternalInput")
    out = nc.dram_tensor("out", [T, C], F32, kind="ExternalOutput")

    with ExitStack() as ctx:
        tc = ctx.enter_context(tile.TileContext(nc))
        const = ctx.enter_context(tc.tile_pool(name="const", bufs=1))
        pss = ctx.enter_context(tc.tile_pool(name="pss", bufs=2, space="PSUM"))
        psy = ctx.enter_context(tc.tile_pool(name="psy", bufs=2, space="PSUM"))
        pq = ctx.enter_context(tc.tile_pool(name="pq", bufs=2, space="PSUM"))
        ppool = ctx.enter_context(tc.tile_pool(name="ppool", bufs=22))
        ypool = ctx.enter_context(tc.tile_pool(name="ypool", bufs=4))
        rpool = ctx.enter_context(tc.tile_pool(name="rpool", bufs=4))
        opool = ctx.enter_context(tc.tile_pool(name="opool", bufs=2))

        xT_sb = const.tile([128, CCH, T], BF16, tag="xT")
        wq_sb = const.tile([128, CCH, 1536], BF16, tag="wq")
        wp_sb = const.tile([128, NPAIR, C], BF16, tag="wp")
        dm_sb = const.tile([128, 128], BF16, tag="dm")
        id_sb = const.tile([128, 128], BF16, tag="id")
        QT_sb = const.tile([128, NPAIR, T], BF16, tag="QT")
        KT_sb = const.tile([128, NPAIR, T], BF16, tag="KT")
        V_sb = const.tile([128, NT, 8 * VW], BF16, tag="V")
        YT_sb = const.tile([128, NPAIR, T], BF16, tag="YT")

        # DMA order: (QKp01 + x half0) per chunk, then V weights, then QKp23,
        # then x half1, then proj weights + constants.  Matches the order
        # compute consumes the data.
        for c in range(CCH):
            nc.sync.dma_start(wq_sb[:, c, 0:512], wqkvT[c * 128:(c + 1) * 128, 0:512])
            nc.sync.dma_start(xT_sb[:, c, 0:1024], xT[c * 128:(c + 1) * 128, 0:1024])
        nc.sync.dma_start(dm_sb[:], dmask[:])
        nc.sync.dma_start(id_sb[:], ident[:])
        for c in range(CCH):
            nc.sync.dma_start(wq_sb[:, c, 1024:1536], wqkvT[c * 128:(c + 1) * 128, 1024:1536])
        for c in range(CCH):
            nc.sync.dma_start(wq_sb[:, c, 512:1024], wqkvT[c * 128:(c + 1) * 128, 512:1024])
        for c in range(CCH):
            nc.sync.dma_start(xT_sb[:, c, 1024:2048], xT[c * 128:(c + 1) * 128, 1024:2048])
        for p in range(NPAIR):
            nc.sync.dma_start(wp_sb[:, p, :], wpT[p * 128:(p + 1) * 128, :])

        vr = V_sb[:].rearrange("p n (h e) -> p n h e", e=VW)
        nc.gpsimd.memset(vr[:, :, :, 64:65], 1.0)

        # ---------- job emitters ----------
        def qk_dst_col(p, half):
            return qcol(p) if half == 0 else kcol(p)

        def emit_qk(p, half, qc):
            # Q^T/K^T pair tile chunk: out [128 feat, 512 t]
            colbase = qk_dst_col(p, half)
            ps = pq.tile([128, 512], F32, tag="pq", name="psqk")
            for c in range(CCH):
                JOBLOG.append(f"qk{p}_{half}_{qc}")
                nc.tensor.matmul(
                    ps[:],
                    wq_sb[:, c, colbase:colbase + 128],
                    xT_sb[:, c, qc * 512:(qc + 1) * 512],
                    start=(c == 0), stop=(c == CCH - 1),
                )
            dst = QT_sb if half == 0 else KT_sb
            nc.vector.tensor_copy(dst[:, p, qc * 512:(qc + 1) * 512], ps[:])

        def emit_qk_quad(qc):
            # chunk-major accumulation of the 4 (pair 0/1) QK jobs for one
            # q-chunk: PE consumes each x chunk as its DMA lands.
            jobs = [(0, 0), (0, 1), (1, 0), (1, 1)]
            tiles = []
            for i, (p, half) in enumerate(jobs):
                pool = pq if i < 2 else pss
                tg = "pq" if i < 2 else "ss"
                ps = pool.tile([128, 512], F32, tag=tg, name=f"q{qc}_{i}")
                tiles.append(ps)
            for c in range(CCH):
                for i, (p, half) in enumerate(jobs):
                    colbase = qk_dst_col(p, half)
                    JOBLOG.append(f"quad{qc}_c{c}")
                    nc.tensor.matmul(
                        tiles[i][:],
                        wq_sb[:, c, colbase:colbase + 128],
                        xT_sb[:, c, qc * 512:(qc + 1) * 512],
                        start=(c == 0), stop=(c == CCH - 1),
                    )
            for i, (p, half) in enumerate(jobs):
                dst = QT_sb if half == 0 else KT_sb
                nc.vector.tensor_copy(
                    dst[:, p, qc * 512:(qc + 1) * 512], tiles[i][:])

        def emit_v(tt):
            # V tile: out [128 t, 512 feat] -> V65 strided
            ps = pq.tile([128, 512], F32, tag="pq", name="psv")
            for c in range(CCH):
                JOBLOG.append(f"v{tt}")
                nc.tensor.matmul(
                    ps[:],
                    xT_sb[:, c, tt * 128:(tt + 1) * 128],
                    wq_sb[:, c, 1024:1536],
                    start=(c == 0), stop=(c == CCH - 1),
                )
            nc.vector.tensor_copy(
                vr[:, tt, :, 0:64],
                ps[:].rearrange("p (h d) -> p h d", d=64),
            )

        def steps_for_pair():
            st = []
            for j in range(NT):
                for g in range((j + 4) // 4):
                    st.append((j, g))
            return st

        def emit_scores(p, j, g, ss, pt, w):
            # head A occupies ss cols [0:w] (PSUM bank pair 0), head B cols
            # [512:512+w] (bank pair 1): the two PE row-groups must not share
            # a PSUM bank.  exp covers both with one strided AP [128, 2, w].
            i0 = 4 * g
            ntile = w // 128
            for ii in range(ntile):
                i = i0 + ii
                JOBLOG.append(f"sc{p}_j{j}_g{g}")
                JOBLOG.append(f"sc{p}_j{j}_g{g}")
                nc.tensor.matmul(
                    ss[:, ii * 128:(ii + 1) * 128],
                    KT_sb[0:64, p, i * 128:(i + 1) * 128],
                    QT_sb[0:64, p, j * 128:(j + 1) * 128],
                    start=True, stop=True,
                )
                nc.tensor.matmul(
                    ss[:, 512 + ii * 128:512 + (ii + 1) * 128],
                    KT_sb[64:128, p, i * 128:(i + 1) * 128],
                    QT_sb[64:128, p, j * 128:(j + 1) * 128],
                    start=True, stop=True,
                )
            fn = AF.Copy if "noexp" in BISECT else AF.Exp
            ssv = ss[:].rearrange("p (b c) -> p b c", b=2)[:, :, 0:w]
            ptv = pt[:].rearrange("p (b c) -> p b c", b=2)[:, :, 0:w]
            nc.scalar.activation(ptv, ssv, fn)
            if i0 + ntile - 1 == j:
                # diagonal tile is last in group: causal mask keep q >= k
                nc.vector.tensor_mul(
                    pt[:, w - 128:w], pt[:, w - 128:w], dm_sb[:])
                nc.vector.tensor_mul(
                    pt[:, 512 + w - 128:512 + w], pt[:, 512 + w - 128:512 + w],
                    dm_sb[:])

        def emit_av_block(p, j, ptiles, ytile):
            # one consecutive accumulation sweep per head (A then B) so each
            # PSUM zero-region (2KB bank) has a single in-flight group
            for hoff, yo in ((0, 0), (1, VW)):
                for i in range(j + 1):
                    g = i // 4
                    pt, w = ptiles[g]
                    ii = i - 4 * g
                    col = (512 if hoff else 0) + ii * 128
                    JOBLOG.append(f"av{p}_j{j}")
                    nc.tensor.matmul(
                        ytile[:, yo:yo + VW],
                        pt[:, col:col + 128],
                        vr[:, i, 2 * p + hoff, :],
                        start=(i == 0), stop=(i == j),
                    )

        def emit_fin_dve(p, j, ytile):
            if "nofin" in BISECT:
                yp = ypool.tile([128, 128], BF16, tag="yp")
                nc.vector.tensor_copy(yp[:, 0:64], ytile[:, 0:64])
                nc.vector.tensor_copy(yp[:, 64:128], ytile[:, VW:VW + 64])
                return yp
            # reciprocal of the two denominator columns, then normalized
            # bf16 eviction of the pair's y tile [128 q, 128 c]
            rc = rpool.tile([128, 2], F32, tag="rc")
            dcol = ytile[:, 0:2 * VW].rearrange(
                "p (h e) -> p h e", e=VW)[:, :, 64:65].rearrange("p h e -> p (h e)")
            nc.vector.reciprocal(rc[:], dcol)
            yp = ypool.tile([128, 128], BF16, tag="yp")
            nc.vector.tensor_scalar_mul(
                yp[:, 0:64], ytile[:, 0:64], rc[:, 0:1])
            nc.vector.tensor_scalar_mul(
                yp[:, 64:128], ytile[:, VW:VW + 64], rc[:, 1:2])
            return yp

        def emit_fin_pe(p, j, yp):
            if "notrn" in BISECT:
                nc.vector.tensor_copy(YT_sb[:, p, j * 128:(j + 1) * 128], yp[:])
                return
            tp = pq.tile([128, 128], BF16, tag="pq", name="tp")
            JOBLOG.append(f"trn{p}_j{j}")
            nc.tensor.transpose(tp[:], yp[:], id_sb[:])
            nc.vector.tensor_copy(YT_sb[:, p, j * 128:(j + 1) * 128], tp[:])

        def emit_proj(tt):
            ot = opool.tile([128, C], F32, tag="ot")
            for oc in range(2):
                po = pq.tile([128, 512], F32, tag="pq", name="po")
                for p in range(NPAIR):
                    JOBLOG.append(f"proj{tt}")
                    nc.tensor.matmul(
                        po[:],
                        YT_sb[:, p, tt * 128:(tt + 1) * 128],
                        wp_sb[:, p, oc * 512:(oc + 1) * 512],
                        start=(p == 0), stop=(p == NPAIR - 1),
                    )
                nc.vector.tensor_copy(ot[:, oc * 512:(oc + 1) * 512], po[:])
            nc.sync.dma_start(out[tt * 128:(tt + 1) * 128, 0:512], ot[:, 0:512])
            nc.sync.dma_start(out[tt * 128:(tt + 1) * 128, 512:1024], ot[:, 512:1024])

        # ---------- schedule: decoupled pairs, guarded fillers ----------
        # Ramp: chunk-paced quad for pairs 0/1, tokens 0:512 (j < 4 usable).
        emit_qk_quad(0)

        fillers = [("quad", 1)]
        fillers += [("v", tt) for tt in range(4)]
        for qc in (2, 3):
            for p in (0, 1):
                for half in (0, 1):
                    fillers.append(("qk", p, half, qc))
        fillers += [("v", 4), ("v", 5)]
        rest = []
        for qc in range(4):
            for p in (2, 3):
                for half in (0, 1):
                    rest.append(("qk", p, half, qc))
        vjobs = list(range(6, NT))
        fi = 0
        for k in range(len(rest) + len(vjobs)):
            if k % 3 == 2 and vjobs:
                fillers.append(("v", vjobs.pop(0)))
            elif fi < len(rest):
                fillers.append(rest[fi]); fi += 1
            elif vjobs:
                fillers.append(("v", vjobs.pop(0)))

        # per-pair state: current qtile, next group to emit, P tiles, phase
        qk_qc = [1, 1, 0, 0]
        jcur = [0] * NPAIR
        gnext = [0] * NPAIR
        ptiles = [[] for _ in range(NPAIR)]
        avq = [None] * NPAIR         # (j, mark, exp_ns) ready for AV block
        pend_fin = [None] * NPAIR
        fin_done = [0] * NPAIR
        proj_next = 0
        fill_i = 0
        v_done = -1
        pe_ns = 0.0
        ss_marks = []

        def take_filler():
            nonlocal fill_i, proj_next, pe_ns, v_done
            if fill_i < len(fillers):
                f = fillers[fill_i]; fill_i += 1
                if f[0] == "qk":
                    _, p, half, qc = f
                    emit_qk(p, half, qc)
                    qk_qc[p] += 0.5
                    pe_ns += 1706
                elif f[0] == "quad":
                    emit_qk_quad(f[1])
                    for p in (0, 1):
                        qk_qc[p] += 1
                    pe_ns += 6827
                else:
                    emit_v(f[1])
                    v_done = max(v_done, f[1])
                    pe_ns += 1706
                return True
            if proj_next < NT and all(fin_done[p] > proj_next for p in range(NPAIR)):
                emit_proj(proj_next)
                proj_next += 1
                pe_ns += 1706
                return True
            return False

        if "noatt" in BISECT:
            jcur = [NT] * NPAIR
            fin_done = [NT] * NPAIR
        while True:
            progressed = False
            for p in range(NPAIR):
                if pend_fin[p] is not None:
                    j, yp = pend_fin[p]
                    emit_fin_pe(p, j, yp)
                    pe_ns += 53
                    pend_fin[p] = None
                    fin_done[p] = j + 1
                    progressed = True
                    if proj_next < NT and all(
                            fin_done[q] > proj_next for q in range(NPAIR)):
                        emit_proj(proj_next)
                        proj_next += 1
                        pe_ns += 1706
                    continue
                if avq[p] is not None:
                    j, mark, exp_ns = avq[p]
                    # last exp of the qtile must be done; V tiles too
                    while pe_ns - mark < exp_ns + 150:
                        if not take_filler():
                            break
                    while v_done < j and fill_i < len(fillers):
                        take_filler()
                    ytile = psy.tile([128, 2 * VW], F32, tag="y",
                                     name=f"yt{p}_{j}")
                    emit_av_block(p, j, ptiles[p], ytile)
                    pe_ns += 2 * (j + 1) * 65 * 0.4167
                    yp = emit_fin_dve(p, j, ytile)
                    pend_fin[p] = (j, yp)
                    avq[p] = None
                    ptiles[p] = []
                    jcur[p] += 1
                    gnext[p] = 0
                    progressed = True
                    continue
                j = jcur[p]
                if j >= NT:
                    take_filler()
                    continue
                if j >= 4 * int(qk_qc[p]):
                    take_filler()
                    continue
                g = gnext[p]
                if len(ss_marks) >= 2:
                    mark, exp_ns = ss_marks[-2]
                    while pe_ns - mark < exp_ns + 150:
                        if not take_filler():
                            break
                w = min(4, j + 1 - 4 * g) * 128
                ss = pss.tile([128, 1024], F32, tag="ss", name="ss")
                ptile = ppool.tile([128, 1024], BF16, tag="pt", name="ptile")
                emit_scores(p, j, g, ss, ptile, w)
                pe_ns += 2 * w * 0.4167
                exp_est = 2 * w * 0.8333 + 185
                ss_marks.append((pe_ns, exp_est))
                ptiles[p].append((ptile, w))
                gnext[p] += 1
                if gnext[p] == (j + 4) // 4:
                    avq[p] = (j, pe_ns, exp_est)
                progressed = True
            if not progressed:
                if not take_filler():
                    break

        while take_filler():
            pass
        while proj_next < NT:
            emit_proj(proj_next)
            proj_next += 1

    return nc


def make_in_maps(x: np.ndarray, w_qkv: np.ndarray, w_proj: np.ndarray):
    bf = ml_dtypes.bfloat16
    scale = np.float32(DH ** -0.5)

    ik = np.arange(128)[:, None]
    iq = np.arange(128)[None, :]
    dmask = (iq >= ik).astype(bf)
    ident = np.eye(128, dtype=bf)

    in_maps = []
    for core in range(NCORES):
        b, g = core // 2, core % 2
        xTb = np.ascontiguousarray(x[b].T).astype(bf)           # [C, T]
        wq = (w_qkv[512 * g: 512 * g + 512] * scale).astype(np.float32)
        wk = w_qkv[1024 + 512 * g: 1024 + 512 * g + 512]
        wv = w_qkv[2048 + 512 * g: 2048 + 512 * g + 512]
        # columns: [Qp0|Kp0|Qp1|Kp1|Qp2|Kp2|Qp3|Kp3|V]
        blocks = []
        for p in range(NPAIR):
            blocks.append(wq[128 * p:128 * (p + 1)])
            blocks.append(wk[128 * p:128 * (p + 1)])
        blocks.append(wv)
        wqkvT = np.ascontiguousarray(
            np.concatenate(blocks, axis=0).T).astype(bf)        # [C, 1536]
        wpT = np.ascontiguousarray(
            w_proj[:, 512 * g: 512 * g + 512].T).astype(bf)     # [512, C]
        in_maps.append({"xT": xTb, "wqkvT": wqkvT, "wpT": wpT,
                        "dmask": dmask, "ident": ident})
    return in_maps


_NC = None


def kernel(x: np.ndarray, w_qkv: np.ndarray, w_proj: np.ndarray,
           _trace: bool = False, _return_raw: bool = False) -> np.ndarray:
    global _NC
    x = np.asarray(x, dtype=np.float32)
    w_qkv = np.asarray(w_qkv, dtype=np.float32)
    w_proj = np.asarray(w_proj, dtype=np.float32)
    if _NC is None:
        _NC = build_program()
    in_maps = make_in_maps(x, w_qkv, w_proj)
    res = run_bass_kernel_spmd(_NC, in_maps, list(range(NCORES)), trace=_trace)
    B = x.shape[0]
    outp = np.empty((B, T, C), dtype=np.float32)
    for b in range(B):
        outp[b] = res.results[2 * b]["out"] + res.results[2 * b + 1]["out"]
    if _return_raw:
        return outp, res
    return outp


# revision 49
# speedup vs baseline: 1.3689x; 1.0435x over previous
"""Causal self-attention (B=4, T=2048, C=1024, H=16) on 8 trn2 NeuronCores.

Sharding: core c -> (batch b = c//2, head-group g = c%2 of 8 heads).
Each core computes qkv projection, causal attention and the proj partial-sum
for its 8 heads on its batch; the host sums the two head-group partials per
batch (row-parallel linear unshard).

Per-core kernel (all PE matmuls bf16, f32 accumulation):
  Scores S^T[k,q] at exact causal q128 granularity, two heads of a pair
  row-group-packed (partitions 0-63 / 64-127).
  exp on ScalarE over pair-packed [128, 2w] PSUM spans.
  AV transposed: y[q,d] = sum_k P^T[k,q].T V[k,d] streams V (n=65 incl. a
  ones column whose output column is the softmax denominator, landing on
  the q partition axis).
  Normalization: DVE reciprocal of the PSUM denominator column + one
  per-partition-scalar multiply fused into the y eviction.
  y pairs transposed back with one PE-transpose per (pair, qtile) into
  Y^T[c, t] for the output projection (k=128 chunks over the 4 pairs).
  DMA order + chunk-major "quad" QKV accumulation pace the input ramp so PE
  starts ~2us in; attention for pairs 0/1 starts once their Q/K q-halves
  are done, with remaining QKV jobs as PE fillers between attention steps.

wqkvT host column layout: [Qp0|Kp0|Qp1|Kp1|Qp2|Kp2|Qp3|Kp3|V] (128 each,
V = 512) so each DMA piece feeds a contiguous phase.
"""

from contextlib import ExitStack

import ml_dtypes
import numpy as np
import orjson

import concourse.bass as bass
import concourse.mybir as mybir
import concourse.tile as tile
from concourse.bass_utils import run_bass_kernel_spmd

BF16 = mybir.dt.bfloat16
F32 = mybir.dt.float32
AF = mybir.ActivationFunctionType

T, C, H, DH = 2048, 1024, 16, 64
NCORES = 8
NPAIR = 4            # head pairs per core (8 heads)
CCH = C // 128       # contraction chunks for qkv
NT = T // 128        # 128-token tiles (k-tiles, q-tiles, t-tiles)
VW = 65              # 64 v-cols + ones column

# --- walrus in this env accepts only ONE sync-wait per instruction: split
# extras onto preceding same-engine NoOps at the BIR-JSON level.
if not getattr(bass.Bass, "_ant_wait_split", False):
    _orig_to_json_bytes = bass.Bass.to_json_bytes

    def _to_json_split_waits(self):
        m = orjson.loads(_orig_to_json_bytes(self))
        for f in m.get("functions", []):
            for bb in f.get("blocks") or []:
                insts = bb.get("instructions") or []
                out, changed = [], False
                for inst in insts:
                    si = inst.get("sync_info")
                    waits = (si or {}).get("on_wait") or []
                    if len(waits) > 1:
                        for j, w in enumerate(waits[:-1]):
                            out.append({
                                "debug": inst.get("debug", 0),
                                "engine": inst["engine"],
                                "ins": [], "outs": [],
                                "name": f"{inst['name']}-sw{j}",
                                "opcode": "NoOp",
                                "sync_info": {"on_wait": [w], "on_update": []},
                            })
                        si["on_wait"] = waits[-1:]
                        changed = True
                    out.append(inst)
                if changed:
                    bb["instructions"] = out
        return orjson.dumps(m)

    bass.Bass.to_json_bytes = _to_json_split_waits
    bass.Bass._ant_wait_split = True


def qcol(p):
    return 256 * p


def kcol(p):
    return 256 * p + 128


JOBLOG = []
BISECT = set()


def build_program() -> bass.Bass:
    JOBLOG.clear()
    nc = bass.Bass()
    xT = nc.dram_tensor("xT", [C, T], BF16, kind="ExternalInput")
    wqkvT = nc.dram_tensor("wqkvT", [C, 1536], BF16, kind="ExternalInput")
    wpT = nc.dram_tensor("wpT", [512, C], BF16, kind="ExternalInput")
    dmask = nc.dram_tensor("dmask", [128, 128], BF16, kind="ExternalInput")
    ident = nc.dram_tensor("ident", [128, 128], BF16, kind="ExternalInput")
    out = nc.dram_tensor("out", [T, C], F32, kind="ExternalOutput")

    with ExitStack() as ctx:
        tc = ctx.enter_context(tile.TileContext(nc))
        const = ctx.enter_context(tc.tile_pool(name="const", bufs=1))
        pss = ctx.enter_context(tc.tile_pool(name="pss", bufs=3, space="PSUM"))
        pq = ctx.enter_context(tc.tile_pool(name="pq", bufs=2, space="PSUM"))
        ppool = ctx.enter_context(tc.tile_pool(name="ppool", bufs=22))
        ypool = ctx.enter_context(tc.tile_pool(name="ypool", bufs=4))
        rpool = ctx.enter_context(tc.tile_pool(name="rpool", bufs=4))
        opool = ctx.enter_context(tc.tile_pool(name="opool", bufs=2))

        xT_sb = const.tile([128, CCH, T], BF16, tag="xT")
        wq_sb = const.tile([128, CCH, 1536], BF16, tag="wq")
        wp_sb = const.tile([128, NPAIR, C], BF16, tag="wp")
        dm_sb = const.tile([128, 128], BF16, tag="dm")
        id_sb = const.tile([128, 128], BF16, tag="id")
        QT_sb = const.tile([128, NPAIR, T], BF16, tag="QT")
        KT_sb = const.tile([128, NPAIR, T], BF16, tag="KT")
        V_sb = const.tile([128, NT, 8 * VW], BF16, tag="V")
        YT_sb = const.tile([128, NPAIR, T], BF16, tag="YT")

        # DMA order: (QKp01 + x half0) per chunk, then V weights, then QKp23,
        # then x half1, then proj weights + constants.  Matches the order
        # compute consumes the data.
        for c in range(CCH):
            nc.sync.dma_start(wq_sb[:, c, 0:512], wqkvT[c * 128:(c + 1) * 128, 0:512])
            nc.sync.dma_start(xT_sb[:, c, 0:1024], xT[c * 128:(c + 1) * 128, 0:1024])
        nc.sync.dma_start(dm_sb[:], dmask[:])
        nc.sync.dma_start(id_sb[:], ident[:])
        for c in range(CCH):
            nc.sync.dma_start(wq_sb[:, c, 1024:1536], wqkvT[c * 128:(c + 1) * 128, 1024:1536])
        for c in range(CCH):
            nc.sync.dma_start(wq_sb[:, c, 512:1024], wqkvT[c * 128:(c + 1) * 128, 512:1024])
        for c in range(CCH):
            nc.sync.dma_start(xT_sb[:, c, 1024:2048], xT[c * 128:(c + 1) * 128, 1024:2048])
        for p in range(NPAIR):
            nc.sync.dma_start(wp_sb[:, p, :], wpT[p * 128:(p + 1) * 128, :])

        vr = V_sb[:].rearrange("p n (h e) -> p n h e", e=VW)
        nc.gpsimd.memset(vr[:, :, :, 64:65], 1.0)

        # ---------- job emitters ----------
        def qk_dst_col(p, half):
            return qcol(p) if half == 0 else kcol(p)

        def emit_qk(p, half, qc):
            # Q^T/K^T pair tile chunk: out [128 feat, 512 t]
            colbase = qk_dst_col(p, half)
            ps = pq.tile([128, 512], F32, tag="pq", name="psqk")
            for c in range(CCH):
                JOBLOG.append(f"qk{p}_{half}_{qc}")
                nc.tensor.matmul(
                    ps[:],
                    wq_sb[:, c, colbase:colbase + 128],
                    xT_sb[:, c, qc * 512:(qc + 1) * 512],
                    start=(c == 0), stop=(c == CCH - 1),
                )
            dst = QT_sb if half == 0 else KT_sb
            nc.vector.tensor_copy(dst[:, p, qc * 512:(qc + 1) * 512], ps[:])

        def emit_qk_quad(qc):
            # chunk-major accumulation of the 4 (pair 0/1) QK jobs for one
            # q-chunk: PE consumes each x chunk as its DMA lands.
            jobs = [(0, 0), (0, 1), (1, 0), (1, 1)]
            tiles = []
            for i, (p, half) in enumerate(jobs):
                pool = pq if i < 2 else pss
                tg = "pq" if i < 2 else "ss"
                ps = pool.tile([128, 512], F32, tag=tg, name=f"q{qc}_{i}")
                tiles.append(ps)
            for c in range(CCH):
                for i, (p, half) in enumerate(jobs):
                    colbase = qk_dst_col(p, half)
                    JOBLOG.append(f"quad{qc}_c{c}")
                    nc.tensor.matmul(
                        tiles[i][:],
                        wq_sb[:, c, colbase:colbase + 128],
                        xT_sb[:, c, qc * 512:(qc + 1) * 512],
                        start=(c == 0), stop=(c == CCH - 1),
                    )
            for i, (p, half) in enumerate(jobs):
                dst = QT_sb if half == 0 else KT_sb
                nc.vector.tensor_copy(
                    dst[:, p, qc * 512:(qc + 1) * 512], tiles[i][:])

        def emit_v(tt):
            # V tile: out [128 t, 512 feat] -> V65 strided
            ps = pq.tile([128, 512], F32, tag="pq", name="psv")
            for c in range(CCH):
                JOBLOG.append(f"v{tt}")
                nc.tensor.matmul(
                    ps[:],
                    xT_sb[:, c, tt * 128:(tt + 1) * 128],
                    wq_sb[:, c, 1024:1536],
                    start=(c == 0), stop=(c == CCH - 1),
                )
            nc.vector.tensor_copy(
                vr[:, tt, :, 0:64],
                ps[:].rearrange("p (h d) -> p h d", d=64),
            )

        def steps_for_pair():
            st = []
            for j in range(NT):
                for g in range((j + 4) // 4):
                    st.append((j, g))
            return st

        def emit_scores(p, j, g, ss, pt, w):
            # head A occupies ss cols [0:w] (PSUM bank pair 0), head B cols
            # [512:512+w] (bank pair 1): the two PE row-groups must not share
            # a PSUM bank.  exp covers both with one strided AP [128, 2, w].
            i0 = 4 * g
            ntile = w // 128
            for ii in range(ntile):
                i = i0 + ii
                JOBLOG.append(f"sc{p}_j{j}_g{g}")
                JOBLOG.append(f"sc{p}_j{j}_g{g}")
                nc.tensor.matmul(
                    ss[:, ii * 128:(ii + 1) * 128],
                    KT_sb[0:64, p, i * 128:(i + 1) * 128],
                    QT_sb[0:64, p, j * 128:(j + 1) * 128],
                    start=True, stop=True,
                )
                nc.tensor.matmul(
                    ss[:, 512 + ii * 128:512 + (ii + 1) * 128],
                    KT_sb[64:128, p, i * 128:(i + 1) * 128],
                    QT_sb[64:128, p, j * 128:(j + 1) * 128],
                    start=True, stop=True,
                )
            fn = AF.Copy if "noexp" in BISECT else AF.Exp
            ssv = ss[:].rearrange("p (b c) -> p b c", b=2)[:, :, 0:w]
            ptv = pt[:].rearrange("p (b c) -> p b c", b=2)[:, :, 0:w]
            nc.scalar.activation(ptv, ssv, fn)
            if i0 + ntile - 1 == j:
                # diagonal tile is last in group: causal mask keep q >= k
                nc.vector.tensor_mul(
                    pt[:, w - 128:w], pt[:, w - 128:w], dm_sb[:])
                nc.vector.tensor_mul(
                    pt[:, 512 + w - 128:512 + w], pt[:, 512 + w - 128:512 + w],
                    dm_sb[:])

        def emit_av_block(p, j, ptiles, ytile):
            # one consecutive accumulation sweep per head (A then B) so each
            # PSUM zero-region (2KB bank) has a single in-flight group
            for hoff, yo in ((0, 0), (1, VW)):
                for i in range(j + 1):
                    g = i // 4
                    pt, w = ptiles[g]
                    ii = i - 4 * g
                    col = (512 if hoff else 0) + ii * 128
                    JOBLOG.append(f"av{p}_j{j}")
                    nc.tensor.matmul(
                        ytile[:, yo:yo + VW],
                        pt[:, col:col + 128],
                        vr[:, i, 2 * p + hoff, :],
                        start=(i == 0), stop=(i == j),
                    )

        def emit_fin_dve(p, j, ytile):
            if "nofin" in BISECT:
                yp = ypool.tile([128, 128], BF16, tag="yp")
                nc.vector.tensor_copy(yp[:, 0:64], ytile[:, 0:64])
                nc.vector.tensor_copy(yp[:, 64:128], ytile[:, VW:VW + 64])
                return yp
            # reciprocal of the two denominator columns, then normalized
            # bf16 eviction of the pair's y tile [128 q, 128 c]
            rc = rpool.tile([128, 2], F32, tag="rc")
            dcol = ytile[:, 0:2 * VW].rearrange(
                "p (h e) -> p h e", e=VW)[:, :, 64:65].rearrange("p h e -> p (h e)")
            nc.vector.reciprocal(rc[:], dcol)
            yp = ypool.tile([128, 128], BF16, tag="yp")
            nc.vector.tensor_scalar_mul(
                yp[:, 0:64], ytile[:, 0:64], rc[:, 0:1])
            nc.vector.tensor_scalar_mul(
                yp[:, 64:128], ytile[:, VW:VW + 64], rc[:, 1:2])
            return yp

        def emit_fin_pe(p, j, yp):
            if "notrn" in BISECT:
                nc.vector.tensor_copy(YT_sb[:, p, j * 128:(j + 1) * 128], yp[:])
                return
            tp = pq.tile([128, 128], BF16, tag="pq", name="tp")
            JOBLOG.append(f"trn{p}_j{j}")
            nc.tensor.transpose(tp[:], yp[:], id_sb[:])
            nc.vector.tensor_copy(YT_sb[:, p, j * 128:(j + 1) * 128], tp[:])

        def emit_proj(tt):
            ot = opool.tile([128, C], F32, tag="ot")
            for oc in range(2):
                po = pq.tile([128, 512], F32, tag="pq", name="po")
                for p in range(NPAIR):
                    JOBLOG.append(f"proj{tt}")
                    nc.tensor.matmul(
                        po[:],
                        YT_sb[:, p, tt * 128:(tt + 1) * 128],
                        wp_sb[:, p, oc * 512:(oc + 1) * 512],
                        start=(p == 0), stop=(p == NPAIR - 1),
                    )
                nc.vector.tensor_copy(ot[:, oc * 512:(oc + 1) * 512], po[:])
            nc.sync.dma_start(out[tt * 128:(tt + 1) * 128, 0:512], ot[:, 0:512])
            nc.sync.dma_start(out[tt * 128:(tt + 1) * 128, 512:1024], ot[:, 512:1024])

        # ---------- schedule: decoupled pairs, guarded fillers ----------
        # Ramp: chunk-paced quads for pairs 0/1 (j < 8 usable).
        emit_qk_quad(0)
        emit_qk_quad(1)

        fillers = []
        fillers += [("v", tt) for tt in range(4)]
        for qc in (2, 3):
            for p in (0, 1):
                for half in (0, 1):
                    fillers.append(("qk", p, half, qc))
        fillers += [("v", 4), ("v", 5)]
        rest = []
        for qc in range(4):
            for p in (2, 3):
                for half in (0, 1):
                    rest.append(("qk", p, half, qc))
        vjobs = list(range(6, NT))
        fi = 0
        for k in range(len(rest) + len(vjobs)):
            if k % 3 == 2 and vjobs:
                fillers.append(("v", vjobs.pop(0)))
            elif fi < len(rest):
                fillers.append(rest[fi]); fi += 1
            elif vjobs:
                fillers.append(("v", vjobs.pop(0)))

        # per-pair state: current qtile, next group to emit, P tiles, phase
        qk_qc = [2, 2, 0, 0]
        jcur = [0] * NPAIR
        gnext = [0] * NPAIR
        ptiles = [[] for _ in range(NPAIR)]
        avq = [None] * NPAIR         # (j, mark, exp_ns) ready for AV block
        pend_fin = [None] * NPAIR
        fin_done = [0] * NPAIR
        proj_next = 0
        fill_i = 0
        v_done = -1
        pe_ns = 0.0
        ss_marks = []

        def take_filler():
            nonlocal fill_i, proj_next, pe_ns, v_done
            if fill_i < len(fillers):
                f = fillers[fill_i]; fill_i += 1
                if f[0] == "qk":
                    _, p, half, qc = f
                    emit_qk(p, half, qc)
                    qk_qc[p] += 0.5
                    pe_ns += 1706
                elif f[0] == "quad":
                    emit_qk_quad(f[1])
                    for p in (0, 1):
                        qk_qc[p] += 1
                    pe_ns += 6827
                else:
                    emit_v(f[1])
                    v_done = max(v_done, f[1])
                    pe_ns += 1706
                return True
            if proj_next < NT and all(fin_done[p] > proj_next for p in range(NPAIR)):
                emit_proj(proj_next)
                proj_next += 1
                pe_ns += 1706
                return True
            return False

        if "noatt" in BISECT:
            jcur = [NT] * NPAIR
            fin_done = [NT] * NPAIR
        while True:
            progressed = False
            for p in range(NPAIR):
                if pend_fin[p] is not None:
                    j, yp = pend_fin[p]
                    emit_fin_pe(p, j, yp)
                    pe_ns += 53
                    pend_fin[p] = None
                    fin_done[p] = j + 1
                    progressed = True
                    if proj_next < NT and all(
                            fin_done[q] > proj_next for q in range(NPAIR)):
                        emit_proj(proj_next)
                        proj_next += 1
                        pe_ns += 1706
                    continue
                if avq[p] is not None:
                    j, mark, exp_ns = avq[p]
                    # last exp of the qtile must be done; V tiles too
                    while pe_ns - mark < exp_ns + 150:
                        if not take_filler():
                            break
                    while v_done < j and fill_i < len(fillers):
                        take_filler()
                    ytile = pq.tile([128, 2 * VW], F32, tag="pq",
                                    name=f"yt{p}_{j}")
                    emit_av_block(p, j, ptiles[p], ytile)
                    pe_ns += 2 * (j + 1) * 65 * 0.4167
                    yp = emit_fin_dve(p, j, ytile)
                    pend_fin[p] = (j, yp)
                    avq[p] = None
                    ptiles[p] = []
                    jcur[p] += 1
                    gnext[p] = 0
                    progressed = True
                    continue
                j = jcur[p]
                if j >= NT:
                    take_filler()
                    continue
                if j >= 4 * int(qk_qc[p]):
                    take_filler()
                    continue
                g = gnext[p]
                if len(ss_marks) >= 3:
                    mark, exp_ns = ss_marks[-3]
                    while pe_ns - mark < exp_ns + 150:
                        if not take_filler():
                            break
                w = min(4, j + 1 - 4 * g) * 128
                ss = pss.tile([128, 1024], F32, tag="ss", name="ss")
                ptile = ppool.tile([128, 1024], BF16, tag="pt", name="ptile")
                emit_scores(p, j, g, ss, ptile, w)
                pe_ns += 2 * w * 0.4167
                exp_est = 2 * w * 0.8333 + 185
                ss_marks.append((pe_ns, exp_est))
                ptiles[p].append((ptile, w))
                gnext[p] += 1
                if gnext[p] == (j + 4) // 4:
                    avq[p] = (j, pe_ns, exp_est)
                progressed = True
            if not progressed:
                if not take_filler():
                    break

        while take_filler():
            pass
        while proj_next < NT:
            emit_proj(proj_next)
            proj_next += 1

    return nc


def make_in_maps(x: np.ndarray, w_qkv: np.ndarray, w_proj: np.ndarray):
    bf = ml_dtypes.bfloat16
    scale = np.float32(DH ** -0.5)

    ik = np.arange(128)[:, None]
    iq = np.arange(128)[None, :]
    dmask = (iq >= ik).astype(bf)
    ident = np.eye(128, dtype=bf)

    in_maps = []
    for core in range(NCORES):
        b, g = core // 2, core % 2
        xTb = np.ascontiguousarray(x[b].T).astype(bf)           # [C, T]
        wq = (w_qkv[512 * g: 512 * g + 512] * scale).astype(np.float32)
        wk = w_qkv[1024 + 512 * g: 1024 + 512 * g + 512]
        wv = w_qkv[2048 + 512 * g: 2048 + 512 * g + 512]
        # columns: [Qp0|Kp0|Qp1|Kp1|Qp2|Kp2|Qp3|Kp3|V]
        blocks = []
        for p in range(NPAIR):
            blocks.append(wq[128 * p:128 * (p + 1)])
            blocks.append(wk[128 * p:128 * (p + 1)])
        blocks.append(wv)
        wqkvT = np.ascontiguousarray(
            np.concatenate(blocks, axis=0).T).astype(bf)        # [C, 1536]
        wpT = np.ascontiguousarray(
            w_proj[:, 512 * g: 512 * g + 512].T).astype(bf)     # [512, C]
        in_maps.append({"xT": xTb, "wqkvT": wqkvT, "wpT": wpT,
                        "dmask": dmask, "ident": ident})
    return in_maps


_NC = None


def kernel(x: np.ndarray, w_qkv: np.ndarray, w_proj: np.ndarray,
           _trace: bool = False, _return_raw: bool = False) -> np.ndarray:
    global _NC
    x = np.asarray(x, dtype=np.float32)
    w_qkv = np.asarray(w_qkv, dtype=np.float32)
    w_proj = np.asarray(w_proj, dtype=np.float32)
    if _NC is None:
        _NC = build_program()
    in_maps = make_in_maps(x, w_qkv, w_proj)
    res = run_bass_kernel_spmd(_NC, in_maps, list(range(NCORES)), trace=_trace)
    B = x.shape[0]
    outp = np.empty((B, T, C), dtype=np.float32)
    for b in range(B):
        outp[b] = res.results[2 * b]["out"] + res.results[2 * b + 1]["out"]
    if _return_raw:
        return outp, res
    return outp


# revision 52
# speedup vs baseline: 1.4015x; 1.0238x over previous
"""Causal self-attention (B=4, T=2048, C=1024, H=16) on 8 trn2 NeuronCores.

Sharding: core c -> (batch b = c//2, head-group g = c%2 of 8 heads).
Each core computes qkv projection, causal attention and the proj partial-sum
for its 8 heads on its batch; the host sums the two head-group partials per
batch (row-parallel linear unshard).

Per-core kernel (all PE matmuls bf16, f32 accumulation):
  Scores S^T[k,q] at exact causal q128 granularity, two heads of a pair
  row-group-packed (partitions 0-63 / 64-127).
  exp on ScalarE over pair-packed [128, 2w] PSUM spans.
  AV transposed: y[q,d] = sum_k P^T[k,q].T V[k,d] streams V (n=65 incl. a
  ones column whose output column is the softmax denominator, landing on
  the q partition axis).
  Normalization: DVE reciprocal of the PSUM denominator column + one
  per-partition-scalar multiply fused into the y eviction.
  y pairs transposed back with one PE-transpose per (pair, qtile) into
  Y^T[c, t] for the output projection (k=128 chunks over the 4 pairs).
  DMA order + chunk-major "quad" QKV accumulation pace the input ramp so PE
  starts ~2us in; attention for pairs 0/1 starts once their Q/K q-halves
  are done, with remaining QKV jobs as PE fillers between attention steps.

wqkvT host column layout: [Qp0|Kp0|Qp1|Kp1|Qp2|Kp2|Qp3|Kp3|V] (128 each,
V = 512) so each DMA piece feeds a contiguous phase.
"""

from contextlib import ExitStack

import ml_dtypes
import numpy as np
import orjson

import concourse.bass as bass
import concourse.mybir as mybir
import concourse.tile as tile
from concourse.bass_utils import run_bass_kernel_spmd

BF16 = mybir.dt.bfloat16
F32 = mybir.dt.float32
AF = mybir.ActivationFunctionType

T, C, H, DH = 2048, 1024, 16, 64
NCORES = 8
NPAIR = 4            # head pairs per core (8 heads)
CCH = C // 128       # contraction chunks for qkv
NT = T // 128        # 128-token tiles (k-tiles, q-tiles, t-tiles)
VW = 65              # 64 v-cols + ones column

# --- walrus in this env accepts only ONE sync-wait per instruction: split
# extras onto preceding same-engine NoOps at the BIR-JSON level.
if not getattr(bass.Bass, "_ant_wait_split", False):
    _orig_to_json_bytes = bass.Bass.to_json_bytes

    def _to_json_split_waits(self):
        m = orjson.loads(_orig_to_json_bytes(self))
        for f in m.get("functions", []):
            for bb in f.get("blocks") or []:
                insts = bb.get("instructions") or []
                out, changed = [], False
                for inst in insts:
                    si = inst.get("sync_info")
                    waits = (si or {}).get("on_wait") or []
                    if len(waits) > 1:
                        for j, w in enumerate(waits[:-1]):
                            out.append({
                                "debug": inst.get("debug", 0),
                                "engine": inst["engine"],
                                "ins": [], "outs": [],
                                "name": f"{inst['name']}-sw{j}",
                                "opcode": "NoOp",
                                "sync_info": {"on_wait": [w], "on_update": []},
                            })
                        si["on_wait"] = waits[-1:]
                        changed = True
                    out.append(inst)
                if changed:
                    bb["instructions"] = out
        return orjson.dumps(m)

    bass.Bass.to_json_bytes = _to_json_split_waits
    bass.Bass._ant_wait_split = True


def qcol(p):
    return 256 * p


def kcol(p):
    return 256 * p + 128


JOBLOG = []
BISECT = set()


def build_program() -> bass.Bass:
    JOBLOG.clear()
    nc = bass.Bass()
    xT = nc.dram_tensor("xT", [C, T], BF16, kind="ExternalInput")
    wqkvT = nc.dram_tensor("wqkvT", [C, 1536], BF16, kind="ExternalInput")
    wpT = nc.dram_tensor("wpT", [512, C], BF16, kind="ExternalInput")
    dmask = nc.dram_tensor("dmask", [128, 128], BF16, kind="ExternalInput")
    ident = nc.dram_tensor("ident", [128, 128], BF16, kind="ExternalInput")
    out = nc.dram_tensor("out", [T, C], F32, kind="ExternalOutput")

    with ExitStack() as ctx:
        tc = ctx.enter_context(tile.TileContext(nc))
        const = ctx.enter_context(tc.tile_pool(name="const", bufs=1))
        pss = ctx.enter_context(tc.tile_pool(name="pss", bufs=3, space="PSUM"))
        pq = ctx.enter_context(tc.tile_pool(name="pq", bufs=2, space="PSUM"))
        ppool = ctx.enter_context(tc.tile_pool(name="ppool", bufs=22))
        ypool = ctx.enter_context(tc.tile_pool(name="ypool", bufs=4))
        rpool = ctx.enter_context(tc.tile_pool(name="rpool", bufs=4))
        opool = ctx.enter_context(tc.tile_pool(name="opool", bufs=2))

        xT_sb = const.tile([128, CCH, T], BF16, tag="xT")
        wq_sb = const.tile([128, CCH, 1536], BF16, tag="wq")
        wp_sb = const.tile([128, NPAIR, C], BF16, tag="wp")
        dm_sb = const.tile([128, 128], BF16, tag="dm")
        id_sb = const.tile([128, 128], BF16, tag="id")
        QT_sb = const.tile([128, NPAIR, T], BF16, tag="QT")
        KT_sb = const.tile([128, NPAIR, T], BF16, tag="KT")
        V_sb = const.tile([128, NT, 8 * VW], BF16, tag="V")
        YT_sb = const.tile([128, NPAIR, T], BF16, tag="YT")

        # DMA order: (QKp01 + x half0) per chunk, then V weights, then QKp23,
        # then x half1, then proj weights + constants.  Matches the order
        # compute consumes the data.
        for c in range(CCH):
            nc.sync.dma_start(wq_sb[:, c, 0:512], wqkvT[c * 128:(c + 1) * 128, 0:512])
            nc.sync.dma_start(xT_sb[:, c, 0:1024], xT[c * 128:(c + 1) * 128, 0:1024])
        nc.sync.dma_start(dm_sb[:], dmask[:])
        nc.sync.dma_start(id_sb[:], ident[:])
        for c in range(CCH):
            nc.sync.dma_start(wq_sb[:, c, 1024:1536], wqkvT[c * 128:(c + 1) * 128, 1024:1536])
        for c in range(CCH):
            nc.sync.dma_start(wq_sb[:, c, 512:1024], wqkvT[c * 128:(c + 1) * 128, 512:1024])
        for c in range(CCH):
            nc.sync.dma_start(xT_sb[:, c, 1024:2048], xT[c * 128:(c + 1) * 128, 1024:2048])
        for p in range(NPAIR):
            nc.sync.dma_start(wp_sb[:, p, :], wpT[p * 128:(p + 1) * 128, :])

        vr = V_sb[:].rearrange("p n (h e) -> p n h e", e=VW)
        nc.gpsimd.memset(vr[:, :, :, 64:65], 1.0)

        # ---------- job emitters ----------
        def qk_dst_col(p, half):
            return qcol(p) if half == 0 else kcol(p)

        def emit_qk(p, half, qc):
            # Q^T/K^T pair tile chunk: out [128 feat, 512 t]
            colbase = qk_dst_col(p, half)
            ps = pq.tile([128, 512], F32, tag="pq", name="psqk")
            for c in range(CCH):
                JOBLOG.append(f"qk{p}_{half}_{qc}")
                nc.tensor.matmul(
                    ps[:],
                    wq_sb[:, c, colbase:colbase + 128],
                    xT_sb[:, c, qc * 512:(qc + 1) * 512],
                    start=(c == 0), stop=(c == CCH - 1),
                )
            dst = QT_sb if half == 0 else KT_sb
            nc.vector.tensor_copy(dst[:, p, qc * 512:(qc + 1) * 512], ps[:])

        def emit_qk_quad(qc):
            # chunk-major accumulation of the 4 (pair 0/1) QK jobs for one
            # q-chunk: PE consumes each x chunk as its DMA lands.
            jobs = [(0, 0), (0, 1), (1, 0), (1, 1)]
            tiles = []
            for i, (p, half) in enumerate(jobs):
                pool = pq if i < 2 else pss
                tg = "pq" if i < 2 else "ss"
                ps = pool.tile([128, 512], F32, tag=tg, name=f"q{qc}_{i}")
                tiles.append(ps)
            for c in range(CCH):
                for i, (p, half) in enumerate(jobs):
                    colbase = qk_dst_col(p, half)
                    JOBLOG.append(f"quad{qc}_c{c}")
                    nc.tensor.matmul(
                        tiles[i][:],
                        wq_sb[:, c, colbase:colbase + 128],
                        xT_sb[:, c, qc * 512:(qc + 1) * 512],
                        start=(c == 0), stop=(c == CCH - 1),
                    )
            for i, (p, half) in enumerate(jobs):
                dst = QT_sb if half == 0 else KT_sb
                nc.vector.tensor_copy(
                    dst[:, p, qc * 512:(qc + 1) * 512], tiles[i][:])

        def emit_v(tt):
            # V tile: out [128 t, 512 feat] -> V65 strided
            ps = pq.tile([128, 512], F32, tag="pq", name="psv")
            for c in range(CCH):
                JOBLOG.append(f"v{tt}")
                nc.tensor.matmul(
                    ps[:],
                    xT_sb[:, c, tt * 128:(tt + 1) * 128],
                    wq_sb[:, c, 1024:1536],
                    start=(c == 0), stop=(c == CCH - 1),
                )
            nc.vector.tensor_copy(
                vr[:, tt, :, 0:64],
                ps[:].rearrange("p (h d) -> p h d", d=64),
            )

        def steps_for_pair():
            st = []
            for j in range(NT):
                for g in range((j + 4) // 4):
                    st.append((j, g))
            return st

        def emit_scores(p, j, g, ss, pt, w):
            # head A occupies ss cols [0:w] (PSUM bank pair 0), head B cols
            # [512:512+w] (bank pair 1): the two PE row-groups must not share
            # a PSUM bank.  exp covers both with one strided AP [128, 2, w].
            i0 = 4 * g
            ntile = w // 128
            for ii in range(ntile):
                i = i0 + ii
                JOBLOG.append(f"sc{p}_j{j}_g{g}")
                JOBLOG.append(f"sc{p}_j{j}_g{g}")
                nc.tensor.matmul(
                    ss[:, ii * 128:(ii + 1) * 128],
                    KT_sb[0:64, p, i * 128:(i + 1) * 128],
                    QT_sb[0:64, p, j * 128:(j + 1) * 128],
                    start=True, stop=True,
                )
                nc.tensor.matmul(
                    ss[:, 512 + ii * 128:512 + (ii + 1) * 128],
                    KT_sb[64:128, p, i * 128:(i + 1) * 128],
                    QT_sb[64:128, p, j * 128:(j + 1) * 128],
                    start=True, stop=True,
                )
            fn = AF.Copy if "noexp" in BISECT else AF.Exp
            ssv = ss[:].rearrange("p (b c) -> p b c", b=2)[:, :, 0:w]
            ptv = pt[:].rearrange("p (b c) -> p b c", b=2)[:, :, 0:w]
            nc.scalar.activation(ptv, ssv, fn)
            if i0 + ntile - 1 == j:
                # diagonal tile is last in group: causal mask keep q >= k
                nc.vector.tensor_mul(
                    pt[:, w - 128:w], pt[:, w - 128:w], dm_sb[:])
                nc.vector.tensor_mul(
                    pt[:, 512 + w - 128:512 + w], pt[:, 512 + w - 128:512 + w],
                    dm_sb[:])

        def emit_av_block(p, j, ptiles, ytile):
            # one consecutive accumulation sweep per head (A then B) so each
            # PSUM zero-region (2KB bank) has a single in-flight group
            for hoff, yo in ((0, 0), (1, VW)):
                for i in range(j + 1):
                    g = i // 4
                    pt, w = ptiles[g]
                    ii = i - 4 * g
                    col = (512 if hoff else 0) + ii * 128
                    JOBLOG.append(f"av{p}_j{j}")
                    nc.tensor.matmul(
                        ytile[:, yo:yo + VW],
                        pt[:, col:col + 128],
                        vr[:, i, 2 * p + hoff, :],
                        start=(i == 0), stop=(i == j),
                    )

        def emit_fin_dve(p, j, ytile):
            if "nofin" in BISECT:
                yp = ypool.tile([128, 128], BF16, tag="yp")
                nc.vector.tensor_copy(yp[:, 0:64], ytile[:, 0:64])
                nc.vector.tensor_copy(yp[:, 64:128], ytile[:, VW:VW + 64])
                return yp
            # reciprocal of the two denominator columns, then normalized
            # bf16 eviction of the pair's y tile [128 q, 128 c]
            rc = rpool.tile([128, 2], F32, tag="rc")
            dcol = ytile[:, 0:2 * VW].rearrange(
                "p (h e) -> p h e", e=VW)[:, :, 64:65].rearrange("p h e -> p (h e)")
            nc.vector.reciprocal(rc[:], dcol)
            yp = ypool.tile([128, 128], BF16, tag="yp")
            nc.vector.tensor_scalar_mul(
                yp[:, 0:64], ytile[:, 0:64], rc[:, 0:1])
            nc.vector.tensor_scalar_mul(
                yp[:, 64:128], ytile[:, VW:VW + 64], rc[:, 1:2])
            return yp

        def emit_fin_pe(p, j, yp):
            if "notrn" in BISECT:
                nc.vector.tensor_copy(YT_sb[:, p, j * 128:(j + 1) * 128], yp[:])
                return
            tp = pq.tile([128, 128], BF16, tag="pq", name="tp")
            JOBLOG.append(f"trn{p}_j{j}")
            nc.tensor.transpose(tp[:], yp[:], id_sb[:])
            nc.vector.tensor_copy(YT_sb[:, p, j * 128:(j + 1) * 128], tp[:])

        def emit_proj(tt):
            ot = opool.tile([128, C], F32, tag="ot")
            for oc in range(2):
                po = pq.tile([128, 512], F32, tag="pq", name="po")
                for p in range(NPAIR):
                    JOBLOG.append(f"proj{tt}")
                    nc.tensor.matmul(
                        po[:],
                        YT_sb[:, p, tt * 128:(tt + 1) * 128],
                        wp_sb[:, p, oc * 512:(oc + 1) * 512],
                        start=(p == 0), stop=(p == NPAIR - 1),
                    )
                nc.vector.tensor_copy(ot[:, oc * 512:(oc + 1) * 512], po[:])
            nc.sync.dma_start(out[tt * 128:(tt + 1) * 128, 0:512], ot[:, 0:512])
            nc.sync.dma_start(out[tt * 128:(tt + 1) * 128, 512:1024], ot[:, 512:1024])

        # ---------- schedule: decoupled pairs, guarded fillers ----------
        # Ramp: chunk-paced quads for pairs 0/1 (j < 8 usable).
        emit_qk_quad(0)
        emit_qk_quad(1)

        fillers = []
        fillers += [("v", tt) for tt in range(4)]
        for qc in (0, 1):
            for p in (2, 3):
                for half in (0, 1):
                    fillers.append(("qk", p, half, qc))
        fillers += [("v", 4), ("v", 5)]
        rest = []
        for qc in (2, 3):
            for p in (0, 1, 2, 3):
                for half in (0, 1):
                    rest.append(("qk", p, half, qc))
        vjobs = list(range(6, NT))
        fi = 0
        for k in range(len(rest) + len(vjobs)):
            if k % 3 == 2 and vjobs:
                fillers.append(("v", vjobs.pop(0)))
            elif fi < len(rest):
                fillers.append(rest[fi]); fi += 1
            elif vjobs:
                fillers.append(("v", vjobs.pop(0)))

        # per-pair state: current qtile, next group to emit, P tiles, phase
        qk_qc = [2, 2, 0, 0]
        jcur = [0] * NPAIR
        gnext = [0] * NPAIR
        ptiles = [[] for _ in range(NPAIR)]
        avq = [None] * NPAIR         # (j, mark, exp_ns) ready for AV block
        pend_fin = [None] * NPAIR
        fin_done = [0] * NPAIR
        proj_next = 0
        fill_i = 0
        v_done = -1
        pe_ns = 0.0
        ss_marks = []

        def take_filler():
            nonlocal fill_i, proj_next, pe_ns, v_done
            if fill_i < len(fillers):
                f = fillers[fill_i]; fill_i += 1
                if f[0] == "qk":
                    _, p, half, qc = f
                    emit_qk(p, half, qc)
                    qk_qc[p] += 0.5
                    pe_ns += 1706
                elif f[0] == "quad":
                    emit_qk_quad(f[1])
                    for p in (0, 1):
                        qk_qc[p] += 1
                    pe_ns += 6827
                else:
                    emit_v(f[1])
                    v_done = max(v_done, f[1])
                    pe_ns += 1706
                return True
            if proj_next < NT and all(fin_done[p] > proj_next for p in range(NPAIR)):
                emit_proj(proj_next)
                proj_next += 1
                pe_ns += 1706
                return True
            return False

        if "noatt" in BISECT:
            jcur = [NT] * NPAIR
            fin_done = [NT] * NPAIR
        while True:
            progressed = False
            for p in range(NPAIR):
                if pend_fin[p] is not None:
                    j, yp = pend_fin[p]
                    emit_fin_pe(p, j, yp)
                    pe_ns += 53
                    pend_fin[p] = None
                    fin_done[p] = j + 1
                    progressed = True
                    if proj_next < NT and all(
                            fin_done[q] > proj_next for q in range(NPAIR)):
                        emit_proj(proj_next)
                        proj_next += 1
                        pe_ns += 1706
                    continue
                if avq[p] is not None:
                    j, mark, exp_ns = avq[p]
                    # last exp of the qtile must be done; V tiles too
                    while pe_ns - mark < exp_ns + 150:
                        if not take_filler():
                            break
                    while v_done < j and fill_i < len(fillers):
                        take_filler()
                    ytile = pq.tile([128, 2 * VW], F32, tag="pq",
                                    name=f"yt{p}_{j}")
                    emit_av_block(p, j, ptiles[p], ytile)
                    pe_ns += 2 * (j + 1) * 65 * 0.4167
                    yp = emit_fin_dve(p, j, ytile)
                    pend_fin[p] = (j, yp)
                    avq[p] = None
                    ptiles[p] = []
                    jcur[p] += 1
                    gnext[p] = 0
                    progressed = True
                    continue
                j = jcur[p]
                if j >= NT:
                    take_filler()
                    continue
                if j >= 4 * int(qk_qc[p]):
                    take_filler()
                    continue
                g = gnext[p]
                if len(ss_marks) >= 3:
                    mark, exp_ns = ss_marks[-3]
                    while pe_ns - mark < exp_ns + 150:
                        if not take_filler():
                            break
                w = min(4, j + 1 - 4 * g) * 128
                ss = pss.tile([128, 1024], F32, tag="ss", name="ss")
                ptile = ppool.tile([128, 1024], BF16, tag="pt", name="ptile")
                emit_scores(p, j, g, ss, ptile, w)
                pe_ns += 2 * w * 0.4167
                exp_est = 2 * w * 0.8333 + 185
                ss_marks.append((pe_ns, exp_est))
                ptiles[p].append((ptile, w))
                gnext[p] += 1
                if gnext[p] == (j + 4) // 4:
                    avq[p] = (j, pe_ns, exp_est)
                progressed = True
            if not progressed:
                if not take_filler():
                    break

        while take_filler():
            pass
        while proj_next < NT:
            emit_proj(proj_next)
            proj_next += 1

    return nc


def make_in_maps(x: np.ndarray, w_qkv: np.ndarray, w_proj: np.ndarray):
    bf = ml_dtypes.bfloat16
    scale = np.float32(DH ** -0.5)

    ik = np.arange(128)[:, None]
    iq = np.arange(128)[None, :]
    dmask = (iq >= ik).astype(bf)
    ident = np.eye(128, dtype=bf)

    in_maps = []
    for core in range(NCORES):
        b, g = core // 2, core % 2
        xTb = np.ascontiguousarray(x[b].T).astype(bf)           # [C, T]
        wq = (w_qkv[512 * g: 512 * g + 512] * scale).astype(np.float32)
        wk = w_qkv[1024 + 512 * g: 1024 + 512 * g + 512]
        wv = w_qkv[2048 + 512 * g: 2048 + 512 * g + 512]
        # columns: [Qp0|Kp0|Qp1|Kp1|Qp2|Kp2|Qp3|Kp3|V]
        blocks = []
        for p in range(NPAIR):
            blocks.append(wq[128 * p:128 * (p + 1)])
            blocks.append(wk[128 * p:128 * (p + 1)])
        blocks.append(wv)
        wqkvT = np.ascontiguousarray(
            np.concatenate(blocks, axis=0).T).astype(bf)        # [C, 1536]
        wpT = np.ascontiguousarray(
            w_proj[:, 512 * g: 512 * g + 512].T).astype(bf)     # [512, C]
        in_maps.append({"xT": xTb, "wqkvT": wqkvT, "wpT": wpT,
                        "dmask": dmask, "ident": ident})
    return in_maps


_NC = None


def kernel(x: np.ndarray, w_qkv: np.ndarray, w_proj: np.ndarray,
           _trace: bool = False, _return_raw: bool = False) -> np.ndarray:
    global _NC
    x = np.asarray(x, dtype=np.float32)
    w_qkv = np.asarray(w_qkv, dtype=np.float32)
    w_proj = np.asarray(w_proj, dtype=np.float32)
    if _NC is None:
        _NC = build_program()
    in_maps = make_in_maps(x, w_qkv, w_proj)
    res = run_bass_kernel_spmd(_NC, in_maps, list(range(NCORES)), trace=_trace)
    B = x.shape[0]
    outp = np.empty((B, T, C), dtype=np.float32)
    for b in range(B):
        outp[b] = res.results[2 * b]["out"] + res.results[2 * b + 1]["out"]
    if _return_raw:
        return outp, res
    return outp
